# revision 59
# baseline (speedup 1.0000x reference)
"""Multi-head attention (B=2, S=2048, E=1024, H=16) on 8 TRN2 NeuronCores.

Sharding: batch x head-group. Core c handles batch c//4 and heads
(c%4)*4 .. +3. Host sums the 4 partials per batch and adds the constant
row bo + tile(bv,H) @ Wo (the V-bias contribution commutes through
softmax normalization).

Algebraic restructure vs the direct formulation:
  - Q/K projections and both biases fold into a single 65x66 matrix
    A2 = [[SCALE*Wk@Wq.T, SCALE*Wk@bq, 0], [0, SCALE*bq.bk, 1]] applied
    to the augmented key-side input [x; 1]: one f32r matmul per head
    produces kt' [66, S] whose rows are [SCALE*Wq Wk.T x; f(s)+c; 1].
    The query side is raw x augmented with [1; g(q)] rows (host-computed
    bf16), so sim = kt'.T @ xq includes every bias cross-term. No
    per-head Q/K projection matmuls, no bias activations.
  - The V projection is eliminated: PV = (sum_s P[s,q] [x_s|1]) then a
    single Wv contraction per q-block. Row 64 of the accumulator U is
    the softmax denominator for free (ones column in the stationary).
  - attention is software-pipelined across the 8 (head, q-half)
    streams: iteration t runs stream t's QK/mask-mul/exp (kc-outer,
    [128,1024] psum tiles written by N=512 matmul pairs - psum-bank
    limit) and stream t-1's PV/U accumulation, giving the in-order PE a
    full iteration of slack on the mul+exp chain. Wo output-projection
    chunks are drip-fed into the pipeline as soon as a q-half's four
    heads are packed.
  - reciprocal via a [1,N] -> [128,N/128] DRAM-bounce repack; rec
    broadcast to 64 partitions via a partition-step-0 DRAM read.
  - mask is consumed qbh-major so the first q-half's strips arrive in
    the first DMAs; bulk input DMAs are kept OFF the scalar (ACT)
    HWDGE queue - a backed-up ring stalls the ACT sequencer and with
    it the casts/exps that gate the pipeline (sync + Pool SWDGE carry
    the bulk instead).
"""
import sys

if "/opt/trn_rl_repo" not in sys.path:
    sys.path.insert(0, "/opt/trn_rl_repo")

from contextlib import ExitStack

import ml_dtypes
import numpy as np

B, S, E = 2, 2048, 1024
H = 16
HD = 64
KD = 64
VD = 64
SCALE = 1.0 / np.float32(np.sqrt(np.float32(KD)))
N_CORES = 8
HPC = H // 4  # heads per core = 4
NKC = S // 128  # 16 k-chunks
QW = 1024  # q-block width
NQB = S // QW  # 2 q-blocks
EW = 2  # k-chunks per exp block

_RUNTIME = {}


def _build_nc(repeat=1):
    import concourse.bass as bass
    import concourse.tile as tile
    from concourse import mybir, bacc

    F32 = mybir.dt.float32
    F32R = mybir.dt.float32r
    BF16 = mybir.dt.bfloat16
    Copy = mybir.ActivationFunctionType.Copy
    Exp = mybir.ActivationFunctionType.Exp

    nc = bacc.Bacc("TRN2")
    wp_d = nc.dram_tensor("wpack", (64, 64), F32, kind="ExternalInput")
    a2_d = nc.dram_tensor("a2", (65, 66), BF16, kind="ExternalInput")
    xq_d = nc.dram_tensor("xq", (66, HPC * S), BF16, kind="ExternalInput")
    xa_d = nc.dram_tensor("xa", (128, NKC * HPC * 65), BF16, kind="ExternalInput")
    mask_d = nc.dram_tensor("maskt", (128, NQB * NKC * QW), BF16, kind="ExternalInput")
    wo_d = nc.dram_tensor("wo", (HPC * VD, E), BF16, kind="ExternalInput")
    out_d = nc.dram_tensor("partial", (S, E), BF16, kind="ExternalOutput")

    with tile.TileContext(nc) as tc:
        with ExitStack() as ctx:
            const = ctx.enter_context(tc.tile_pool(name="const", bufs=1))
            ptp = ctx.enter_context(tc.tile_pool(name="ptp", bufs=10))
            usp = ctx.enter_context(tc.tile_pool(name="usp", bufs=2))
            otp = ctx.enter_context(tc.tile_pool(name="otp", bufs=2))
            ot2p = ctx.enter_context(tc.tile_pool(name="ot2p", bufs=2))
            denp = ctx.enter_context(tc.tile_pool(name="denp", bufs=3))
            wst = ctx.enter_context(tc.tile_pool(name="wst", bufs=3))
            drp = ctx.enter_context(tc.tile_pool(name="drp", bufs=2, space="DRAM"))
            smp = ctx.enter_context(tc.tile_pool(name="smp", bufs=3, space="PSUM"))
            upp = ctx.enter_context(tc.tile_pool(name="upp", bufs=1, space="PSUM"))

            # ---- constant loads, spread over the two HWDGE queues (sync,
            # scalar) for early-needed data and the Pool SWDGE queue for
            # late-needed bulk, all in consumption order: A2 + query-side x
            # (kt' phase feeds from xq rows 0..64), then the first q-half's
            # mask strips, PV stationary, second-half mask, wv + Wo. ----
            # NOTE: no bulk loads on the scalar (ACT) queue — a backed-up
            # HWDGE ring stalls the ACT engine's sequencer and with it the
            # kt' casts that gate the whole pipeline.
            a2_sb = const.tile([65, 66], BF16, tag="a2")
            nc.sync.dma_start(out=a2_sb, in_=a2_d[:, :])
            xq_sb = const.tile([66, HPC * S], BF16, tag="xq")
            for hh in range(4):
                sl = slice(hh * S, (hh + 1) * S)
                nc.sync.dma_start(out=xq_sb[:, sl], in_=xq_d[:, sl])
            mask_sb = const.tile([128, NQB * NKC * QW], BF16, tag="mask")
            nmd = NQB * NKC
            for md in range(nmd // 2):
                eng = nc.sync if md % 2 == 0 else nc.gpsimd
                eng.dma_start(
                    out=mask_sb[:, md * QW : (md + 1) * QW],
                    in_=mask_d[:, md * QW : (md + 1) * QW],
                )
            xa_sb = const.tile([128, NKC * HPC * 65], BF16, tag="xa")
            nc.gpsimd.dma_start(out=xa_sb, in_=xa_d[:, :])
            wp_sb = const.tile([64, 64], F32, tag="wp")
            nc.gpsimd.dma_start(
                out=wp_sb.bitcast(F32R), in_=wp_d[:, :].bitcast(F32R)
            )
            wv_sb = wp_sb
            for md in range(nmd // 2, nmd):
                eng = nc.gpsimd if md % 2 == 0 else nc.sync
                eng.dma_start(
                    out=mask_sb[:, md * QW : (md + 1) * QW],
                    in_=mask_d[:, md * QW : (md + 1) * QW],
                )
            wo_sb = []
            for g in range(2):
                t = const.tile([128, E], BF16, tag=f"wo{g}")
                nc.gpsimd.dma_start(out=t, in_=wo_d[g * 128 : (g + 1) * 128, :])
                wo_sb.append(t)

            for rep in range(repeat):
                # kt' = A2.T @ [x;1] per head (bf16, from xq rows 0..64).
                # Emitted lazily - head h's matmuls go just before the first
                # stream that needs them, so the in-order PE never parks
                # stream 0's QK behind kt' work that waits on later xq DMAs.
                kt = [None] * HPC

                def emit_kt(h):
                    t = const.tile([66, S], BF16, tag=f"kt{h}", name=f"kt{h}_r{rep}")
                    for half in range(2):
                        ps = smp.tile([128, QW], F32, tag="sm")
                        for e2 in range(2):
                            co = h * S + half * QW + e2 * 512
                            nc.tensor.matmul(
                                ps[0:66, e2 * 512 : (e2 + 1) * 512],
                                a2_sb,
                                xq_sb[0:65, co : co + 512],
                                start=True,
                                stop=True,
                            )
                        nc.scalar.activation(
                            t[:, half * QW : (half + 1) * QW], ps[0:66, :], Copy
                        )
                    kt[h] = t

                # ---- phase 2: attention, software-pipelined across the 8
                # (head, q-half) streams: iteration t runs stream t's
                # QK/mask/exp and stream t-1's PV/U accumulation, so the PE
                # never waits on the mul+exp chain (a full iteration of
                # slack). Stream t-1's normalize/pack tail is emitted at
                # iteration t+1 after the first block so its U cast has
                # drained by the time the PE reaches the Wv matmuls. ----
                ot2 = [
                    ot2p.tile([128, S], BF16, tag="ot2", name=f"ot2_g{g}_r{rep}")
                    for g in range(2)
                ]
                NST = HPC * NQB
                nblk = NKC // EW
                ptls = {}
                ups = {}
                wo_pending = []

                def emit_wo(qc):
                    wo_ps = smp.tile([128, QW], F32, tag="sm")
                    for gi in range(2):
                        for e2 in range(2):
                            nc.tensor.matmul(
                                wo_ps[:, e2 * 512 : (e2 + 1) * 512],
                                ot2[gi][:, qc * 128 : (qc + 1) * 128],
                                wo_sb[gi][:, e2 * 512 : (e2 + 1) * 512],
                                start=(gi == 0),
                                stop=(gi == 1),
                            )
                    ost = wst.tile([128, E], BF16, tag="wst")
                    if qc % 2 == 0:
                        nc.scalar.activation(ost, wo_ps, Copy)
                    else:
                        nc.vector.tensor_copy(ost, wo_ps)
                    eng = nc.sync if qc % 2 == 0 else nc.gpsimd
                    eng.dma_start(out=out_d[qc * 128 : (qc + 1) * 128, :], in_=ost)

                def finish_stream(tp):
                    hp, qbhp = tp % HPC, tp // HPC
                    up = ups.pop(tp)
                    us = usp.tile([65, QW], F32R, tag="us")
                    nc.scalar.activation(us, up, Copy)
                    pv = smp.tile([128, QW], F32, tag="sm")
                    for e2 in range(2):
                        nc.tensor.matmul(
                            pv[0:64, e2 * 512 : (e2 + 1) * 512],
                            wv_sb.bitcast(F32R),
                            us[0:64, e2 * 512 : (e2 + 1) * 512],
                            start=True,
                            stop=True,
                        )
                    # denominator (us row 64) -> reciprocal -> broadcast
                    dden = drp.tile([1, QW], F32, tag="dden")
                    nc.sync.dma_start(out=dden, in_=us[64:65, :].bitcast(F32))
                    dpk = denp.tile([128, QW // 128], F32, tag="dpk")
                    nc.sync.dma_start(
                        out=dpk,
                        in_=dden.rearrange("a (p f) -> (a p) f", p=128),
                    )
                    rpk = denp.tile([128, QW // 128], F32, tag="rpk")
                    nc.vector.reciprocal(rpk, dpk)
                    drec = drp.tile([1, QW], F32, tag="drec")
                    nc.sync.dma_start(
                        out=drec.rearrange("a (p f) -> (a p) f", p=128),
                        in_=rpk,
                    )
                    recb = denp.tile([64, QW], F32, tag="recb")
                    nc.sync.dma_start(
                        out=recb,
                        in_=bass.AP(
                            tensor=drec.tensor,
                            offset=drec.offset,
                            ap=[[0, 64]] + [list(a) for a in drec.ap[1:]],
                        ),
                    )
                    ot = otp.tile([64, QW], BF16, tag="ot")
                    nc.vector.tensor_mul(ot, pv[0:64, :], recb)
                    # pack into the head-pair group tile (partition-shift DMA;
                    # Pool queue so the ACT sequencer never blocks on DGE)
                    nc.gpsimd.dma_start(
                        out=ot2[hp // 2][
                            (hp % 2) * 64 : (hp % 2) * 64 + 64,
                            qbhp * QW : (qbhp + 1) * QW,
                        ],
                        in_=ot,
                    )

                for h in range(HPC):
                    emit_kt(h)
                for t in range(NST + 2):
                    for kb in range(nblk):
                        if kb == 0 and t >= 2:
                            finish_stream(t - 2)
                            del ptls[t - 2]
                            if (t - 2) % HPC == HPC - 1:
                                qb_done = (t - 2) // HPC
                                wo_pending.extend(
                                    range(qb_done * (QW // 128), (qb_done + 1) * (QW // 128))
                                )
                        if t < NST:
                            h, qbh = t % HPC, t // HPC
                            pt2 = ptp.tile([128, EW * QW], BF16, tag="pt")
                            for j in range(EW):
                                kc = kb * EW + j
                                sm = smp.tile([128, QW], F32, tag="sm")
                                for e2 in range(2):
                                    qo = h * S + qbh * QW + e2 * 512
                                    nc.tensor.matmul(
                                        sm[:, e2 * 512 : (e2 + 1) * 512],
                                        kt[h][:, kc * 128 : (kc + 1) * 128],
                                        xq_sb[:, qo : qo + 512],
                                        start=True,
                                        stop=True,
                                    )
                                moff = (qbh * NKC + kc) * QW
                                nc.vector.tensor_mul(
                                    pt2[:, j * QW : (j + 1) * QW],
                                    sm,
                                    mask_sb[:, moff : moff + QW],
                                )
                            nc.scalar.activation(pt2, pt2, Exp)
                            ptls.setdefault(t, []).append(pt2)
                        if 1 <= t <= NST:
                            tp = t - 1
                            hp = tp % HPC
                            if kb == 0:
                                ups[tp] = upp.tile(
                                    [65, QW], F32, tag="u", name=f"up{tp}_r{rep}"
                                )
                            up = ups[tp]
                            pt2b = ptls[tp][kb]
                            for j in range(EW):
                                kc = kb * EW + j
                                for e2 in range(2):
                                    nc.tensor.matmul(
                                        up[:, e2 * 512 : (e2 + 1) * 512],
                                        xa_sb[
                                            :,
                                            kc * (HPC * 65)
                                            + hp * 65 : kc * (HPC * 65)
                                            + (hp + 1) * 65,
                                        ],
                                        pt2b[
                                            :,
                                            j * QW + e2 * 512 : j * QW + (e2 + 1) * 512,
                                        ],
                                        start=(kc == 0),
                                        stop=(kc == NKC - 1),
                                    )
                        if wo_pending and kb % 2 == 1:
                            emit_wo(wo_pending.pop(0))
                while wo_pending:
                    emit_wo(wo_pending.pop(0))

    nc.finalize()
    return nc


def _build_runner(repeat=1):
    """Compile once. Returns an object with:
    - prep(in_maps): host arrays -> device-resident committed args
    - make_zeros(): device-side zero output buffers (donated per exec)
    - exec_device(args): one bass execution -> sharded partials (blocked)
    - reduce_device(partials): on-device cross-core sum -> (B*S, E)
    - run(in_maps): full host->host pipeline (correctness path)
    """
    import jax
    import jax.numpy as jnp
    import numpy as _np
    from jax.experimental.shard_map import shard_map
    from jax.sharding import Mesh, NamedSharding, PartitionSpec

    from concourse import mybir
    from concourse.bass2jax import (
        _bass_exec_p,
        fast_dispatch_compile,
        install_neuronx_cc_hook,
        partition_id_tensor,
    )

    nc = _build_nc(repeat=repeat)
    install_neuronx_cc_hook()
    partition_name = nc.partition_id_tensor.name if nc.partition_id_tensor else None

    replicated = {"maskt", "wpack", "a2"}

    in_names, out_names, out_avals, out_shapes, out_dtypes = [], [], [], [], []
    per_core_shapes = {}
    for alloc in nc.m.functions[0].allocations:
        if not isinstance(alloc, mybir.MemoryLocationSet):
            continue
        name = alloc.memorylocations[0].name
        per_core_shapes[name] = (
            tuple(alloc.tensor_shape), mybir.dt.np(alloc.dtype),
        )
        if alloc.kind == "ExternalInput":
            if name != partition_name:
                in_names.append(name)
        elif alloc.kind == "ExternalOutput":
            out_names.append(name)
            shape = tuple(alloc.tensor_shape)
            dtype = mybir.dt.np(alloc.dtype)
            out_avals.append(jax.core.ShapedArray(shape, dtype))
            out_shapes.append(shape)
            out_dtypes.append(dtype)

    n_params = len(in_names)
    n_outs = len(out_names)
    all_in_names = list(in_names) + list(out_names)
    if partition_name is not None:
        all_in_names.append(partition_name)
    donate = tuple(range(n_params, n_params + n_outs))

    def _body(*args):
        operands = list(args)
        if partition_name is not None:
            operands.append(partition_id_tensor())
        outs = _bass_exec_p.bind(
            *operands,
            out_avals=tuple(out_avals),
            in_names=tuple(all_in_names),
            out_names=tuple(out_names),
            lowering_input_output_aliases=(),
            sim_require_finite=True,
            sim_require_nnan=True,
            nc=nc,
        )
        return tuple(outs)

    devices = jax.devices()[:N_CORES]
    mesh = Mesh(_np.asarray(devices), ("core",))
    shard0 = NamedSharding(mesh, PartitionSpec("core"))
    srepl = NamedSharding(mesh, PartitionSpec())
    in_specs = tuple(
        PartitionSpec() if name in replicated else PartitionSpec("core")
        for name in in_names
    ) + (PartitionSpec("core"),) * n_outs
    out_specs = (PartitionSpec("core"),) * n_outs

    # AOT-compile with bass_effect suppressed so per-call dispatch takes the
    # C++ fast path.
    shape_args = []
    for name in in_names:
        shp, dt = per_core_shapes[name]
        if name in replicated:
            shape_args.append(jax.ShapeDtypeStruct(shp, dt, sharding=srepl))
        else:
            shape_args.append(jax.ShapeDtypeStruct(
                (N_CORES * shp[0],) + shp[1:], dt, sharding=shard0))
    for shp, dt in zip(out_shapes, out_dtypes):
        shape_args.append(jax.ShapeDtypeStruct(
            (N_CORES * shp[0],) + shp[1:], dt, sharding=shard0))

    def _compile():
        jitted = jax.jit(
            shard_map(
                _body, mesh=mesh, in_specs=in_specs, out_specs=out_specs,
                check_rep=False,
            ),
            donate_argnums=donate,
            keep_unused=True,
        )
        return jitted.lower(*shape_args).compile()

    sharded = fast_dispatch_compile(_compile)

    _zeros = jax.jit(
        lambda: tuple(
            jnp.zeros((N_CORES * s[0], *s[1:]), d)
            for s, d in zip(out_shapes, out_dtypes)
        ),
        out_shardings=(shard0,) * n_outs,
    )

    _reduce = jax.jit(
        lambda p: p.reshape(B, 4, S, E).sum(axis=1).reshape(B * S, E),
        out_shardings=shard0,
    )

    def prep(in_maps):
        args = []
        for name in in_names:
            if name in replicated:
                arr = _np.asarray(in_maps[0][name])
                args.append(jax.device_put(arr, srepl))
            else:
                arr = _np.concatenate(
                    [_np.asarray(m[name]) for m in in_maps], axis=0
                )
                args.append(jax.device_put(arr, shard0))
        return args

    def make_zeros():
        return _zeros()

    def exec_device(args, zeros=None):
        if zeros is None:
            zeros = _zeros()
        outs = sharded(*args, *zeros)
        return jax.block_until_ready(outs[0])

    def exec_async(args, zeros):
        return sharded(*args, *zeros)[0]

    def reduce_device(partials):
        return jax.block_until_ready(_reduce(partials))

    def run(in_maps):
        partials = exec_device(prep(in_maps))
        return _np.asarray(reduce_device(partials))  # (B*S, E)

    class R:
        pass

    r = R()
    r.prep = prep
    r.make_zeros = make_zeros
    r.exec_device = exec_device
    r.exec_async = exec_async
    r.reduce_device = reduce_device
    r.run = run
    return r


def _runtime(repeat=1):
    if repeat not in _RUNTIME:
        _RUNTIME[repeat] = _build_runner(repeat=repeat)
    return _RUNTIME[repeat]


def make_in_maps(x, mask, Wq, bq, Wk, bk, Wv, bv, Wo, bo):
    bf16 = ml_dtypes.bfloat16
    x = np.asarray(x, np.float32)
    Wq = np.asarray(Wq, np.float32)
    bq = np.asarray(bq, np.float32)
    Wk = np.asarray(Wk, np.float32)
    bk = np.asarray(bk, np.float32)
    Wv = np.asarray(Wv, np.float32)

    # A2 folds Q/K projections, both biases, and SCALE into one matrix
    # applied to [x_s; 1]; the query side carries [x_q; 1; g(q)].
    A2 = np.zeros((65, 66), np.float32)
    A2[0:64, 0:64] = SCALE * (Wk @ Wq.T)
    A2[0:64, 64] = SCALE * (Wk @ bq)
    A2[64, 64] = SCALE * float(bq @ bk)
    A2[64, 65] = 1.0
    wqbk = Wq @ bk  # g(q) = SCALE * x_q @ wqbk
    wpack = np.ascontiguousarray(Wv, np.float32)

    m = np.asarray(mask, np.float32).T  # [s, q]
    # device consumes [128, (qbh, kc, qw)] blocks, qbh-major
    maskT = np.ascontiguousarray(
        m.reshape(NKC, 128, NQB, QW).transpose(1, 2, 0, 3).reshape(128, NQB * NKC * QW)
    ).astype(bf16)

    in_maps = []
    for c in range(N_CORES):
        b = c // 4
        h0 = (c % 4) * HPC
        r0 = h0 * HD
        xh = x[b][:, r0 : r0 + HPC * HD]  # [S, 256]

        xq = np.empty((66, HPC * S), np.float32)
        xa = np.empty((NKC, 128, HPC * 65), np.float32)
        for i in range(HPC):
            xi = xh[:, i * HD : (i + 1) * HD]  # [S, 64]
            sl = slice(i * S, (i + 1) * S)
            xq[0:64, sl] = xi.T
            xq[64, sl] = 1.0
            xq[65, sl] = SCALE * (xi @ wqbk)
            xa[:, :, i * 65 : i * 65 + 64] = xi.reshape(NKC, 128, HD)
            xa[:, :, i * 65 + 64] = 1.0

        wo = np.ascontiguousarray(
            np.asarray(Wo, np.float32)[r0 : r0 + HPC * VD, :]
        ).astype(bf16)
        in_maps.append(
            {
                "wpack": wpack,
                "a2": A2.astype(bf16),
                "xq": np.ascontiguousarray(xq).astype(bf16),
                "xa": np.ascontiguousarray(
                    xa.reshape(NKC, 128, HPC * 65).transpose(1, 0, 2).reshape(
                        128, NKC * HPC * 65
                    )
                ).astype(bf16),
                "maskt": maskT,
                "wo": wo,
            }
        )
    return in_maps


def kernel(x, mask, Wq, bq, Wk, bk, Wv, bv, Wo, bo):
    r = _runtime()
    in_maps = make_in_maps(x, mask, Wq, bq, Wk, bk, Wv, bv, Wo, bo)
    flat = r.run(in_maps)  # (B*S, E), per-batch partials already summed
    Wo32 = np.asarray(Wo, np.float32)
    crow = np.asarray(bo, np.float32) + np.tile(np.asarray(bv, np.float32), H) @ Wo32
    out = flat.reshape(B, S, E) + crow[None, None, :]
    return out.astype(np.float32)


# revision 61
# speedup vs baseline: 1.1566x; 1.1566x over previous
"""Multi-head attention (B=2, S=2048, E=1024, H=16) on 8 TRN2 NeuronCores.

Sharding: batch x head-group. Core c handles batch c//4 and heads
(c%4)*4 .. +3. Host sums the 4 partials per batch and adds the constant
row bo + tile(bv,H) @ Wo (the V-bias contribution commutes through
softmax normalization).

Algebraic restructure vs the direct formulation:
  - Q/K projections and both biases fold into a single 65x66 matrix
    A2 = [[SCALE*Wk@Wq.T, SCALE*Wk@bq, 0], [0, SCALE*bq.bk, 1]] applied
    to the augmented key-side input [x; 1]: one f32r matmul per head
    produces kt' [66, S] whose rows are [SCALE*Wq Wk.T x; f(s)+c; 1].
    The query side is raw x augmented with [1; g(q)] rows (host-computed
    bf16), so sim = kt'.T @ xq includes every bias cross-term. No
    per-head Q/K projection matmuls, no bias activations.
  - The V projection is eliminated: PV = (sum_s P[s,q] [x_s|1]) then a
    single Wv contraction per q-block. Row 64 of the accumulator U is
    the softmax denominator for free (ones column in the stationary).
  - attention is software-pipelined across the 8 (head, q-half)
    streams: iteration t runs stream t's QK/mask-mul/exp (kc-outer,
    [128,1024] psum tiles written by N=512 matmul pairs - psum-bank
    limit) and stream t-1's PV/U accumulation, giving the in-order PE a
    full iteration of slack on the mul+exp chain. Wo output-projection
    chunks are drip-fed into the pipeline as soon as a q-half's four
    heads are packed.
  - reciprocal via a [1,N] -> [128,N/128] DRAM-bounce repack; rec
    broadcast to 64 partitions via a partition-step-0 DRAM read.
  - mask is consumed qbh-major so the first q-half's strips arrive in
    the first DMAs; bulk input DMAs are kept OFF the scalar (ACT)
    HWDGE queue - a backed-up ring stalls the ACT sequencer and with
    it the casts/exps that gate the pipeline (sync + Pool SWDGE carry
    the bulk instead).
"""
import sys

if "/opt/trn_rl_repo" not in sys.path:
    sys.path.insert(0, "/opt/trn_rl_repo")

from contextlib import ExitStack

import ml_dtypes
import numpy as np

B, S, E = 2, 2048, 1024
H = 16
HD = 64
KD = 64
VD = 64
SCALE = 1.0 / np.float32(np.sqrt(np.float32(KD)))
N_CORES = 8
HPC = H // 4  # heads per core = 4
NKC = S // 128  # 16 k-chunks
QW = 1024  # q-block width
NQB = S // QW  # 2 q-blocks
EW = 2  # k-chunks per exp block

_RUNTIME = {}


def _build_nc(repeat=1):
    import concourse.bass as bass
    import concourse.tile as tile
    from concourse import mybir, bacc

    F32 = mybir.dt.float32
    F32R = mybir.dt.float32r
    BF16 = mybir.dt.bfloat16
    Copy = mybir.ActivationFunctionType.Copy
    Exp = mybir.ActivationFunctionType.Exp

    nc = bacc.Bacc("TRN2")
    wp_d = nc.dram_tensor("wpack", (64, 64), F32, kind="ExternalInput")
    a2_d = nc.dram_tensor("a2", (65, 66), BF16, kind="ExternalInput")
    xq_d = nc.dram_tensor("xq", (66, HPC * S), BF16, kind="ExternalInput")
    xa_d = nc.dram_tensor("xa", (128, NKC * HPC * 65), BF16, kind="ExternalInput")
    mask_d = nc.dram_tensor("maskt", (128, NQB * NKC * QW), BF16, kind="ExternalInput")
    wo_d = nc.dram_tensor("wo", (HPC * VD, E), BF16, kind="ExternalInput")
    out_d = nc.dram_tensor("partial", (S, E), BF16, kind="ExternalOutput")

    with tile.TileContext(nc) as tc:
        with ExitStack() as ctx:
            const = ctx.enter_context(tc.tile_pool(name="const", bufs=1))
            ptp = ctx.enter_context(tc.tile_pool(name="ptp", bufs=10))
            usp = ctx.enter_context(tc.tile_pool(name="usp", bufs=2))
            otp = ctx.enter_context(tc.tile_pool(name="otp", bufs=2))
            ot2p = ctx.enter_context(tc.tile_pool(name="ot2p", bufs=2))
            denp = ctx.enter_context(tc.tile_pool(name="denp", bufs=3))
            wst = ctx.enter_context(tc.tile_pool(name="wst", bufs=3))
            drp = ctx.enter_context(tc.tile_pool(name="drp", bufs=2, space="DRAM"))
            smp = ctx.enter_context(tc.tile_pool(name="smp", bufs=3, space="PSUM"))
            upp = ctx.enter_context(tc.tile_pool(name="upp", bufs=1, space="PSUM"))

            # ---- constant loads, spread over the two HWDGE queues (sync,
            # scalar) for early-needed data and the Pool SWDGE queue for
            # late-needed bulk, all in consumption order: A2 + query-side x
            # (kt' phase feeds from xq rows 0..64), then the first q-half's
            # mask strips, PV stationary, second-half mask, wv + Wo. ----
            # NOTE: no bulk loads on the scalar (ACT) queue — a backed-up
            # HWDGE ring stalls the ACT engine's sequencer and with it the
            # kt' casts that gate the whole pipeline.
            a2_sb = const.tile([65, 66], BF16, tag="a2")
            nc.sync.dma_start(out=a2_sb, in_=a2_d[:, :])
            xq_sb = const.tile([66, HPC * S], BF16, tag="xq")
            for hh in range(4):
                sl = slice(hh * S, (hh + 1) * S)
                nc.sync.dma_start(out=xq_sb[:, sl], in_=xq_d[:, sl])
            mask_sb = const.tile([128, NQB * NKC * QW], BF16, tag="mask")
            nmd = NQB * NKC
            for md in range(nmd // 2):
                eng = nc.sync if md % 2 == 0 else nc.gpsimd
                eng.dma_start(
                    out=mask_sb[:, md * QW : (md + 1) * QW],
                    in_=mask_d[:, md * QW : (md + 1) * QW],
                )
            xa_sb = const.tile([128, NKC * HPC * 65], BF16, tag="xa")
            nc.gpsimd.dma_start(out=xa_sb, in_=xa_d[:, :])
            wp_sb = const.tile([64, 64], F32, tag="wp")
            nc.gpsimd.dma_start(
                out=wp_sb.bitcast(F32R), in_=wp_d[:, :].bitcast(F32R)
            )
            wv_sb = wp_sb
            for md in range(nmd // 2, nmd):
                eng = nc.gpsimd if md % 2 == 0 else nc.sync
                eng.dma_start(
                    out=mask_sb[:, md * QW : (md + 1) * QW],
                    in_=mask_d[:, md * QW : (md + 1) * QW],
                )
            wo_sb = []
            for g in range(2):
                t = const.tile([128, E], BF16, tag=f"wo{g}")
                nc.gpsimd.dma_start(out=t, in_=wo_d[g * 128 : (g + 1) * 128, :])
                wo_sb.append(t)

            for rep in range(repeat):
                # kt' = A2.T @ [x;1] per head (bf16, from xq rows 0..64).
                # Emitted lazily - head h's matmuls go just before the first
                # stream that needs them, so the in-order PE never parks
                # stream 0's QK behind kt' work that waits on later xq DMAs.
                kt = [None] * HPC

                def emit_kt(h):
                    t = const.tile([66, S], BF16, tag=f"kt{h}", name=f"kt{h}_r{rep}")
                    for half in range(2):
                        ps = smp.tile([128, QW], F32, tag="sm")
                        for e2 in range(2):
                            co = h * S + half * QW + e2 * 512
                            nc.tensor.matmul(
                                ps[0:66, e2 * 512 : (e2 + 1) * 512],
                                a2_sb,
                                xq_sb[0:65, co : co + 512],
                                start=True,
                                stop=True,
                            )
                        nc.scalar.activation(
                            t[:, half * QW : (half + 1) * QW], ps[0:66, :], Copy
                        )
                    kt[h] = t

                for h in range(HPC):
                    emit_kt(h)

                # ---- phase 2: attention, software-pipelined across the 8
                # (head, q-half) streams: iteration t runs stream t's
                # QK/mask/exp and stream t-1's PV/U accumulation, so the PE
                # never waits on the mul+exp chain (a full iteration of
                # slack). Stream t-1's normalize/pack tail is emitted at
                # iteration t+1 after the first block so its U cast has
                # drained by the time the PE reaches the Wv matmuls. ----
                ot2 = [
                    ot2p.tile([128, S], BF16, tag="ot2", name=f"ot2_g{g}_r{rep}")
                    for g in range(2)
                ]
                NST = HPC * NQB
                nblk = NKC // EW
                ptls = {}
                ups = {}
                wo_pending = []

                def emit_wo(qc):
                    wo_ps = smp.tile([128, QW], F32, tag="sm")
                    for gi in range(2):
                        for e2 in range(2):
                            nc.tensor.matmul(
                                wo_ps[:, e2 * 512 : (e2 + 1) * 512],
                                ot2[gi][:, qc * 128 : (qc + 1) * 128],
                                wo_sb[gi][:, e2 * 512 : (e2 + 1) * 512],
                                start=(gi == 0),
                                stop=(gi == 1),
                            )
                    ost = wst.tile([128, E], BF16, tag="wst")
                    if qc % 2 == 0:
                        nc.scalar.activation(ost, wo_ps, Copy)
                    else:
                        nc.vector.tensor_copy(ost, wo_ps)
                    eng = nc.sync if qc % 2 == 0 else nc.gpsimd
                    eng.dma_start(out=out_d[qc * 128 : (qc + 1) * 128, :], in_=ost)

                def finish_stream(tp):
                    hp, qbhp = tp % HPC, tp // HPC
                    up = ups.pop(tp)
                    us = usp.tile([65, QW], F32R, tag="us")
                    nc.scalar.activation(us, up, Copy)
                    pv = smp.tile([128, QW], F32, tag="sm")
                    for e2 in range(2):
                        nc.tensor.matmul(
                            pv[0:64, e2 * 512 : (e2 + 1) * 512],
                            wv_sb.bitcast(F32R),
                            us[0:64, e2 * 512 : (e2 + 1) * 512],
                            start=True,
                            stop=True,
                        )
                    # denominator (us row 64) -> reciprocal -> broadcast
                    dden = drp.tile([1, QW], F32, tag="dden")
                    nc.sync.dma_start(out=dden, in_=us[64:65, :].bitcast(F32))
                    dpk = denp.tile([128, QW // 128], F32, tag="dpk")
                    nc.sync.dma_start(
                        out=dpk,
                        in_=dden.rearrange("a (p f) -> (a p) f", p=128),
                    )
                    rpk = denp.tile([128, QW // 128], F32, tag="rpk")
                    nc.vector.reciprocal(rpk, dpk)
                    drec = drp.tile([1, QW], F32, tag="drec")
                    nc.sync.dma_start(
                        out=drec.rearrange("a (p f) -> (a p) f", p=128),
                        in_=rpk,
                    )
                    recb = denp.tile([64, QW], F32, tag="recb")
                    nc.sync.dma_start(
                        out=recb,
                        in_=bass.AP(
                            tensor=drec.tensor,
                            offset=drec.offset,
                            ap=[[0, 64]] + [list(a) for a in drec.ap[1:]],
                        ),
                    )
                    ot = otp.tile([64, QW], BF16, tag="ot")
                    nc.vector.tensor_mul(ot, pv[0:64, :], recb)
                    # pack into the head-pair group tile (partition-shift DMA;
                    # Pool queue so the ACT sequencer never blocks on DGE)
                    nc.gpsimd.dma_start(
                        out=ot2[hp // 2][
                            (hp % 2) * 64 : (hp % 2) * 64 + 64,
                            qbhp * QW : (qbhp + 1) * QW,
                        ],
                        in_=ot,
                    )

                for t in range(NST + 2):
                    for kb in range(nblk):
                        if kb == 0 and t >= 2:
                            finish_stream(t - 2)
                            del ptls[t - 2]
                            if (t - 2) % HPC == HPC - 1:
                                qb_done = (t - 2) // HPC
                                wo_pending.extend(
                                    range(qb_done * (QW // 128), (qb_done + 1) * (QW // 128))
                                )
                        if t < NST:
                            h, qbh = t % HPC, t // HPC
                            pt2 = ptp.tile([128, EW * QW], BF16, tag="pt")
                            for j in range(EW):
                                kc = kb * EW + j
                                sm = smp.tile([128, QW], F32, tag="sm")
                                for e2 in range(2):
                                    qo = h * S + qbh * QW + e2 * 512
                                    nc.tensor.matmul(
                                        sm[:, e2 * 512 : (e2 + 1) * 512],
                                        kt[h][:, kc * 128 : (kc + 1) * 128],
                                        xq_sb[:, qo : qo + 512],
                                        start=True,
                                        stop=True,
                                    )
                                moff = (qbh * NKC + kc) * QW
                                nc.vector.tensor_mul(
                                    pt2[:, j * QW : (j + 1) * QW],
                                    sm,
                                    mask_sb[:, moff : moff + QW],
                                )
                            nc.scalar.activation(pt2, pt2, Exp)
                            ptls.setdefault(t, []).append(pt2)
                        if 1 <= t <= NST:
                            tp = t - 1
                            hp = tp % HPC
                            if kb == 0:
                                ups[tp] = upp.tile(
                                    [65, QW], F32, tag="u", name=f"up{tp}_r{rep}"
                                )
                            up = ups[tp]
                            pt2b = ptls[tp][kb]
                            for j in range(EW):
                                kc = kb * EW + j
                                for e2 in range(2):
                                    nc.tensor.matmul(
                                        up[:, e2 * 512 : (e2 + 1) * 512],
                                        xa_sb[
                                            :,
                                            kc * (HPC * 65)
                                            + hp * 65 : kc * (HPC * 65)
                                            + (hp + 1) * 65,
                                        ],
                                        pt2b[
                                            :,
                                            j * QW + e2 * 512 : j * QW + (e2 + 1) * 512,
                                        ],
                                        start=(kc == 0),
                                        stop=(kc == NKC - 1),
                                    )
                        if wo_pending and kb % 2 == 1:
                            emit_wo(wo_pending.pop(0))
                while wo_pending:
                    emit_wo(wo_pending.pop(0))

    nc.finalize()
    return nc


def _build_runner(repeat=1):
    """Compile once. Returns an object with:
    - prep(in_maps): host arrays -> device-resident committed args
    - make_zeros(): device-side zero output buffers (donated per exec)
    - exec_device(args): one bass execution -> sharded partials (blocked)
    - reduce_device(partials): on-device cross-core sum -> (B*S, E)
    - run(in_maps): full host->host pipeline (correctness path)
    """
    import jax
    import jax.numpy as jnp
    import numpy as _np
    from jax.experimental.shard_map import shard_map
    from jax.sharding import Mesh, NamedSharding, PartitionSpec

    from concourse import mybir
    from concourse.bass2jax import (
        _bass_exec_p,
        fast_dispatch_compile,
        install_neuronx_cc_hook,
        partition_id_tensor,
    )

    nc = _build_nc(repeat=repeat)
    install_neuronx_cc_hook()
    partition_name = nc.partition_id_tensor.name if nc.partition_id_tensor else None

    replicated = {"maskt", "wpack", "a2"}

    in_names, out_names, out_avals, out_shapes, out_dtypes = [], [], [], [], []
    per_core_shapes = {}
    for alloc in nc.m.functions[0].allocations:
        if not isinstance(alloc, mybir.MemoryLocationSet):
            continue
        name = alloc.memorylocations[0].name
        per_core_shapes[name] = (
            tuple(alloc.tensor_shape), mybir.dt.np(alloc.dtype),
        )
        if alloc.kind == "ExternalInput":
            if name != partition_name:
                in_names.append(name)
        elif alloc.kind == "ExternalOutput":
            out_names.append(name)
            shape = tuple(alloc.tensor_shape)
            dtype = mybir.dt.np(alloc.dtype)
            out_avals.append(jax.core.ShapedArray(shape, dtype))
            out_shapes.append(shape)
            out_dtypes.append(dtype)

    n_params = len(in_names)
    n_outs = len(out_names)
    all_in_names = list(in_names) + list(out_names)
    if partition_name is not None:
        all_in_names.append(partition_name)
    donate = tuple(range(n_params, n_params + n_outs))

    def _body(*args):
        operands = list(args)
        if partition_name is not None:
            operands.append(partition_id_tensor())
        outs = _bass_exec_p.bind(
            *operands,
            out_avals=tuple(out_avals),
            in_names=tuple(all_in_names),
            out_names=tuple(out_names),
            lowering_input_output_aliases=(),
            sim_require_finite=True,
            sim_require_nnan=True,
            nc=nc,
        )
        return tuple(outs)

    devices = jax.devices()[:N_CORES]
    mesh = Mesh(_np.asarray(devices), ("core",))
    shard0 = NamedSharding(mesh, PartitionSpec("core"))
    srepl = NamedSharding(mesh, PartitionSpec())
    in_specs = tuple(
        PartitionSpec() if name in replicated else PartitionSpec("core")
        for name in in_names
    ) + (PartitionSpec("core"),) * n_outs
    out_specs = (PartitionSpec("core"),) * n_outs

    # AOT-compile with bass_effect suppressed so per-call dispatch takes the
    # C++ fast path.
    shape_args = []
    for name in in_names:
        shp, dt = per_core_shapes[name]
        if name in replicated:
            shape_args.append(jax.ShapeDtypeStruct(shp, dt, sharding=srepl))
        else:
            shape_args.append(jax.ShapeDtypeStruct(
                (N_CORES * shp[0],) + shp[1:], dt, sharding=shard0))
    for shp, dt in zip(out_shapes, out_dtypes):
        shape_args.append(jax.ShapeDtypeStruct(
            (N_CORES * shp[0],) + shp[1:], dt, sharding=shard0))

    def _compile():
        jitted = jax.jit(
            shard_map(
                _body, mesh=mesh, in_specs=in_specs, out_specs=out_specs,
                check_rep=False,
            ),
            donate_argnums=donate,
            keep_unused=True,
        )
        return jitted.lower(*shape_args).compile()

    sharded = fast_dispatch_compile(_compile)

    _zeros = jax.jit(
        lambda: tuple(
            jnp.zeros((N_CORES * s[0], *s[1:]), d)
            for s, d in zip(out_shapes, out_dtypes)
        ),
        out_shardings=(shard0,) * n_outs,
    )

    _reduce = jax.jit(
        lambda p: p.reshape(B, 4, S, E).sum(axis=1).reshape(B * S, E),
        out_shardings=shard0,
    )

    def prep(in_maps):
        args = []
        for name in in_names:
            if name in replicated:
                arr = _np.asarray(in_maps[0][name])
                args.append(jax.device_put(arr, srepl))
            else:
                arr = _np.concatenate(
                    [_np.asarray(m[name]) for m in in_maps], axis=0
                )
                args.append(jax.device_put(arr, shard0))
        return args

    def make_zeros():
        return _zeros()

    def exec_device(args, zeros=None):
        if zeros is None:
            zeros = _zeros()
        outs = sharded(*args, *zeros)
        return jax.block_until_ready(outs[0])

    def exec_async(args, zeros):
        return sharded(*args, *zeros)[0]

    def reduce_device(partials):
        return jax.block_until_ready(_reduce(partials))

    def run(in_maps):
        partials = exec_device(prep(in_maps))
        return _np.asarray(reduce_device(partials))  # (B*S, E)

    class R:
        pass

    r = R()
    r.prep = prep
    r.make_zeros = make_zeros
    r.exec_device = exec_device
    r.exec_async = exec_async
    r.reduce_device = reduce_device
    r.run = run
    return r


def _runtime(repeat=1):
    if repeat not in _RUNTIME:
        _RUNTIME[repeat] = _build_runner(repeat=repeat)
    return _RUNTIME[repeat]


def make_in_maps(x, mask, Wq, bq, Wk, bk, Wv, bv, Wo, bo):
    bf16 = ml_dtypes.bfloat16
    x = np.asarray(x, np.float32)
    Wq = np.asarray(Wq, np.float32)
    bq = np.asarray(bq, np.float32)
    Wk = np.asarray(Wk, np.float32)
    bk = np.asarray(bk, np.float32)
    Wv = np.asarray(Wv, np.float32)

    # A2 folds Q/K projections, both biases, and SCALE into one matrix
    # applied to [x_s; 1]; the query side carries [x_q; 1; g(q)].
    A2 = np.zeros((65, 66), np.float32)
    A2[0:64, 0:64] = SCALE * (Wk @ Wq.T)
    A2[0:64, 64] = SCALE * (Wk @ bq)
    A2[64, 64] = SCALE * float(bq @ bk)
    A2[64, 65] = 1.0
    wqbk = Wq @ bk  # g(q) = SCALE * x_q @ wqbk
    wpack = np.ascontiguousarray(Wv, np.float32)

    m = np.asarray(mask, np.float32).T  # [s, q]
    # device consumes [128, (qbh, kc, qw)] blocks, qbh-major
    maskT = np.ascontiguousarray(
        m.reshape(NKC, 128, NQB, QW).transpose(1, 2, 0, 3).reshape(128, NQB * NKC * QW)
    ).astype(bf16)

    in_maps = []
    for c in range(N_CORES):
        b = c // 4
        h0 = (c % 4) * HPC
        r0 = h0 * HD
        xh = x[b][:, r0 : r0 + HPC * HD]  # [S, 256]

        xq = np.empty((66, HPC * S), np.float32)
        xa = np.empty((NKC, 128, HPC * 65), np.float32)
        for i in range(HPC):
            xi = xh[:, i * HD : (i + 1) * HD]  # [S, 64]
            sl = slice(i * S, (i + 1) * S)
            xq[0:64, sl] = xi.T
            xq[64, sl] = 1.0
            xq[65, sl] = SCALE * (xi @ wqbk)
            xa[:, :, i * 65 : i * 65 + 64] = xi.reshape(NKC, 128, HD)
            xa[:, :, i * 65 + 64] = 1.0

        wo = np.ascontiguousarray(
            np.asarray(Wo, np.float32)[r0 : r0 + HPC * VD, :]
        ).astype(bf16)
        in_maps.append(
            {
                "wpack": wpack,
                "a2": A2.astype(bf16),
                "xq": np.ascontiguousarray(xq).astype(bf16),
                "xa": np.ascontiguousarray(
                    xa.reshape(NKC, 128, HPC * 65).transpose(1, 0, 2).reshape(
                        128, NKC * HPC * 65
                    )
                ).astype(bf16),
                "maskt": maskT,
                "wo": wo,
            }
        )
    return in_maps


def kernel(x, mask, Wq, bq, Wk, bk, Wv, bv, Wo, bo):
    r = _runtime()
    in_maps = make_in_maps(x, mask, Wq, bq, Wk, bk, Wv, bv, Wo, bo)
    flat = r.run(in_maps)  # (B*S, E), per-batch partials already summed
    Wo32 = np.asarray(Wo, np.float32)
    crow = np.asarray(bo, np.float32) + np.tile(np.asarray(bv, np.float32), H) @ Wo32
    out = flat.reshape(B, S, E) + crow[None, None, :]
    return out.astype(np.float32)
